# revision 3
# baseline (speedup 1.0000x reference)
"""Trainium2 Bass kernel for nn_Attention_55130200211640 (v2).

GQA attention block: q/k/v projections + RoPE (theta=1e6) + non-causal
softmax attention (16 q-heads, 4 kv-heads, head_dim 128) + output
projection. B=4, S=2048, HID=2048, fp32 I/O.

Sharding: (batch x 4) x (kv-group-half x 2) = 8 cores, tensor-parallel
over heads within a batch. Each core computes, for its batch, the full
2048-token sequence restricted to 2 of the 4 kv groups (= 8 of the 16
q heads): q/k/v projections, RoPE, attention, and a PARTIAL o_proj
(contraction over its 8 heads only). The host adds the two partial y's
per batch (all-reduce done on host; sim time measures one core's NEFF).

Everything computes in bf16 on the PE (psum f32), which both halves
SBUF/DMA footprint (no DRAM bounce at all: X^T stays resident in SBUF
through the whole kernel) and keeps 1 cycle/row matmul throughput.

Per-core dataflow ("contraction-on-partition" layouts everywhere):
  preamble: X^T [hid,2048] bf16 resident; K^T [d,2|S] and V [j,256]
            computed+roped, Q^T head 0.
  slots:    16 slots = (8 heads) x (2 query halves of 1024). Per slot:
            S^T[j,i] = K^T_g . Q^T_h on PE -> exp on ACT (scale folded)
            -> E bf16; U^T[d,i] = sum_j V E (psum-accumulated);
            Z[i] = sum_j E via 128 stationary-E matmuls with a [128,1]
            ones moving operand (1 row each - partition reduction at
            free-size cost); Z cols -> free-axis reduce (DVE) -> recip
            -> PE transpose -> per-block gpsimd partition_broadcast ->
            one DVE mul U*(1/Z) -> OT bf16. Next head's q-projection
            matmuls + rope are interleaved into the slot as PE filler
            so the PE never idles while ACT paces the exps.
  o_proj:   y[i,o] = sum_h OT_h . wo_h, psum-accumulated over the 8
            heads, 8-bank rotation, evicted via ACT/DVE/Pool copies.
"""

import numpy as np

B, S, HID = 4, 2048, 2048
H, KV, D = 16, 4, 128
N_CORES = 8
HC = 8                 # heads per core
KVC = 2                # kv groups per core
CT = HID // 128        # contraction tiles
JT = S // 128          # key tiles
SCALE = 1.0 / float(np.sqrt(D))

_cache = {}


def _emit(nc, tc, io):
    import concourse.mybir as mybir
    from collections import deque
    from contextlib import ExitStack

    F32 = mybir.dt.float32
    BF16 = mybir.dt.bfloat16
    Exp = mybir.ActivationFunctionType.Exp
    AxX = mybir.AxisListType.X
    Add = mybir.AluOpType.add

    xt_d, cosk_d, sinkm_d, wqt_d, wkt_d, wvt_d, wot_d, ones_d, ident_d, y_d = io

    ctx = ExitStack()

    # ---------------- persistent SBUF tiles (left heap) ----------------
    const_pool = ctx.enter_context(tc.tile_pool(name="const", bufs=1, side="left"))
    ones_t = const_pool.tile([128, 1], BF16)
    ident_t = const_pool.tile([128, 128], F32)
    COS = const_pool.tile([128, S], F32)
    SINM = const_pool.tile([128, S], F32)

    dram_pool = ctx.enter_context(tc.tile_pool(name="drp", bufs=1, space="DRAM"))
    kv_pool = ctx.enter_context(tc.tile_pool(name="kv", bufs=1, side="left"))
    KT = kv_pool.tile([128, KVC, S], BF16)        # [d, g, j]
    VV = kv_pool.tile([128, JT, KVC * 128], BF16)  # [j, jt, g*128+d]
    q_pool = ctx.enter_context(tc.tile_pool(name="qt", bufs=1, side="left"))
    QT = q_pool.tile([128, HC, S], BF16)          # [d, h, i]
    o_pool = ctx.enter_context(tc.tile_pool(name="ot", bufs=1, side="left"))
    OT = o_pool.tile([128, HC, S], BF16)          # [d, h, i]

    # X^T resident for the whole projection span. Opened last on the left
    # heap so it can be closed (LIFO) mid-emission to make room for wo.
    x_ctx = ExitStack()
    x_pool = x_ctx.enter_context(tc.tile_pool(name="xp", bufs=1, side="left"))
    X = x_pool.tile([128, CT, S], BF16)           # [hid%128, ct, tok]

    w_pool = ctx.enter_context(tc.tile_pool(name="wp", bufs=2, side="right"))
    st_pool = ctx.enter_context(tc.tile_pool(name="st", bufs=1, side="right"))
    e_pool = ctx.enter_context(tc.tile_pool(name="ep", bufs=3, side="right"))
    rz_pool = ctx.enter_context(tc.tile_pool(name="rz", bufs=1, side="right"))

    # Persistent psum pool for projection chunks: 1 bank, used by the
    # q-projection filler during the slots as well.
    pq_ctx = ExitStack()
    p_q = pq_ctx.enter_context(
        tc.tile_pool(name="p_q", bufs=1, space="PSUM", side="left"))

    def rope(ps, c0, n, dst):
        """RoPE a [128, n] psum tile (layout [d, pos], positions c0:c0+n)
        -> bf16 SBUF dst. rotate_half is a cross-partition half-swap; the
        sign lives in SINM (rows 0:64 pre-negated on the host)."""
        tmp = st_pool.tile([128, 512], F32, tag="tmp", bufs=2, name="tmp")
        stage = st_pool.tile([128, 512], F32, tag="stage", bufs=2, name="stage")
        nc.vector.tensor_mul(stage[0:64, 0:n], ps[64:128, :],
                             SINM[0:64, c0:c0 + n])
        nc.vector.tensor_mul(stage[64:128, 0:n], ps[0:64, :],
                             SINM[64:128, c0:c0 + n])
        nc.vector.tensor_mul(tmp[:, 0:n], ps[:], COS[:, c0:c0 + n])
        nc.vector.tensor_add(dst, stage[:, 0:n], tmp[:, 0:n])

    # ---------------- preamble: K, V, Q(0) projections ----------------
    # DMA priority: feed the PE within ~4.5us and keep the X stream just
    # ahead of the 256-token preamble chunk walk (6.8us compute / 2.9us
    # DMA per chunk).
    wk_ts = []
    wk_t0 = w_pool.tile([128, CT, 128], BF16, tag="w", bufs=3, name="wk_t")
    nc.sync.dma_start(wk_t0[:], wkt_d[0])
    wk_ts.append(wk_t0)
    nc.sync.dma_start(COS[:, 0:512], cosk_d[:, 0:512])
    nc.sync.dma_start(SINM[:, 0:512], sinkm_d[:, 0:512])
    nc.sync.dma_start(X[:, :, 0:256], xt_d[:, :, 0:256])
    wk_t1 = w_pool.tile([128, CT, 128], BF16, tag="w", bufs=3, name="wk_t")
    nc.sync.dma_start(wk_t1[:], wkt_d[1])
    wk_ts.append(wk_t1)
    wv_t = w_pool.tile([128, CT, 256], BF16, tag="wv", bufs=1, name="wv_t")
    nc.sync.dma_start(wv_t[:], wvt_d[:])
    nc.sync.dma_start(X[:, :, 256:512], xt_d[:, :, 256:512])
    nc.sync.dma_start(COS[:, 512:1024], cosk_d[:, 512:1024])
    nc.sync.dma_start(SINM[:, 512:1024], sinkm_d[:, 512:1024])
    nc.sync.dma_start(X[:, :, 512:768], xt_d[:, :, 512:768])
    nc.sync.dma_start(COS[:, 1024:2048], cosk_d[:, 1024:2048])
    nc.sync.dma_start(SINM[:, 1024:2048], sinkm_d[:, 1024:2048])
    for tch in range(3, 8):
        t0 = tch * 256
        nc.sync.dma_start(X[:, :, t0:t0 + 256], xt_d[:, :, t0:t0 + 256])
    wq_t0 = w_pool.tile([128, CT, 128], BF16, tag="w", bufs=3, name="wq_t")
    nc.sync.dma_start(wq_t0[:], wqt_d[0])
    nc.sync.dma_start(ones_t[:], ones_d[:])
    nc.sync.dma_start(ident_t[:], ident_d[:])

    with tc.tile_pool(name="p_pre", bufs=1, space="PSUM", side="right") as p_pre:
        for tch in range(8):
            j0 = tch * 256
            for g in range(KVC):
                ps = p_pre.tile([128, 512], F32, tag="c", bufs=6, name="ps_pre")
                for ct in range(CT):
                    nc.tensor.matmul(ps[:, 0:256], wk_ts[g][:, ct, :],
                                     X[:, ct, j0:j0 + 256],
                                     start=(ct == 0), stop=(ct == CT - 1))
                rope(ps[:, 0:256], j0, 256, KT[:, g, j0:j0 + 256])
            for jt in range(tch * 2, tch * 2 + 2):
                ps = p_pre.tile([128, 512], F32, tag="c", bufs=6, name="ps_pre")
                for ct in range(CT):
                    nc.tensor.matmul(ps[:, 0:256],
                                     X[:, ct, jt * 128:(jt + 1) * 128],
                                     wv_t[:, ct, :],
                                     start=(ct == 0), stop=(ct == CT - 1))
                nc.scalar.copy(VV[:, jt, :], ps[:, 0:256])
        # Q projection head 0.
        for qc in range(4):
            i0 = qc * 512
            ps = p_pre.tile([128, 512], F32, tag="c", bufs=6, name="ps_pre")
            for ct in range(CT):
                nc.tensor.matmul(ps[:], wq_t0[:, ct, :],
                                 X[:, ct, i0:i0 + 512],
                                 start=(ct == 0), stop=(ct == CT - 1))
            rope(ps, i0, 512, QT[:, 0, i0:i0 + 512])

    # ---------------- q-projection filler machinery ----------------
    filler = deque()

    wq_pref = {}

    def prefetch_wq(h):
        if h < HC and h not in wq_pref:
            wq_t = w_pool.tile([128, CT, 128], BF16, tag="w", bufs=3,
                               name="wq_t")
            nc.sync.dma_start(wq_t[:], wqt_d[h])
            wq_pref[h] = wq_t

    def push_qproj(h):
        """Queue head h's q-projection as small PE filler pieces."""
        prefetch_wq(h)
        state = {"w": wq_pref.pop(h)}

        for qc in range(4):
            def open_chunk(qc=qc):
                ps = p_q.tile([128, 512], F32, tag="q", bufs=1, name="ps_q")
                state["ps"] = ps
            filler.append((False, open_chunk))
            for c4 in range(4):
                def mm_piece(qc=qc, c4=c4):
                    ps = state["ps"]
                    for ct in range(c4 * 4, c4 * 4 + 4):
                        nc.tensor.matmul(
                            ps[:], state["w"][:, ct, :],
                            X[:, ct, qc * 512:qc * 512 + 512],
                            start=(ct == 0), stop=(ct == CT - 1))
                filler.append((True, mm_piece))

            def rope_piece(qc=qc):
                rope(state["ps"], qc * 512, 512, QT[:, h, qc * 512:qc * 512 + 512])
            filler.append((False, rope_piece))

    # Fixed-position pacing: a qproj chunk is 6 pieces (open, 4 mm, rope)
    # emitted at jts {0..4, 8} and {8..12, ...} of each slot, i.e. two
    # chunks per slot, finishing well clear of the slot boundary so the
    # single qproj psum bank and the DVE rope never collide with the
    # slot-end usb/normalize chain.
    PIECE_POINTS = (0, 1, 2, 3, 4, 5, 8, 9, 10, 11, 12, 13)

    def drain_at(point):
        k = 0
        while filler and k < PIECE_POINTS.count(point):
            filler.popleft()[1]()
            k += 1

    def flush_filler():
        while filler:
            filler.popleft()[1]()

    # ---------------- attention slots ----------------
    wo_ts = []
    prefilled = set()
    ys_ev = [0]

    def oproj_group(key):
        tt, ob = key
        o0 = ob * 512
        ps = p_q.tile([128, 512], F32, tag="q", bufs=1, name="ps_q")
        for hh in range(HC):
            nc.tensor.matmul(ps[:], OT[:, hh, tt * 128:(tt + 1) * 128],
                             wo_ts[hh][:, o0:o0 + 512],
                             start=(hh == 0), stop=(hh == HC - 1))
        yt = st_pool.tile([128, 512], F32, tag="yt8", bufs=2, name="yt")
        if ys_ev[0] % 2 == 0:
            nc.scalar.copy(yt[:], ps[:])
        else:
            nc.vector.tensor_copy(yt[:], ps[:])
        ys_ev[0] += 1
        nc.sync.dma_start(y_d[tt * 128:(tt + 1) * 128, o0:o0 + 512], yt[:])
        prefilled.add(key)

    og_iter = iter([(0, 0), (0, 1), (0, 2), (0, 3), (1, 0)])
    with (
        tc.tile_pool(name="p_s", bufs=1, space="PSUM", side="right") as p_s,
        tc.tile_pool(name="p_u", bufs=1, space="PSUM", side="right") as p_u,
        tc.tile_pool(name="p_z", bufs=1, space="PSUM", side="right") as p_z,
    ):
        for h in range(HC):
            g = h // 4
            prefetch_wq(h + 2)
            if h + 1 < HC:
                push_qproj(h + 1)
            for u in range(2):
                i0 = u * 1024
                U_ps = p_u.tile([128, 1024], F32, tag="U", bufs=1, name="ps_U")
                Z_ps = p_z.tile([128, 512], F32, tag="Z", bufs=1, name="ps_Z")
                Es = {}

                def score(jt):
                    ps = p_s.tile([128, 1024], F32, tag="S", bufs=2, name="ps_S")
                    kt_sl = KT[:, g, jt * 128:(jt + 1) * 128]
                    nc.tensor.matmul(ps[:, 0:512], kt_sl,
                                     QT[:, h, i0:i0 + 512],
                                     start=True, stop=True)
                    nc.tensor.matmul(ps[:, 512:1024], kt_sl,
                                     QT[:, h, i0 + 512:i0 + 1024],
                                     start=True, stop=True)
                    E = e_pool.tile([128, 1024], BF16, tag="e", bufs=3, name="E")
                    nc.scalar.activation(E[:], ps[:], Exp, scale=SCALE)
                    Es[jt] = E

                def av_z(jt):
                    # Z column layout: [jt-half, ib, jt%8] so the first
                    # half's reduce can run mid-slot and the next slot's
                    # writes never collide with this slot's late reduce.
                    E = Es.pop(jt)
                    v_sl = VV[:, jt, g * 128:(g + 1) * 128]
                    st, sp = (jt == 0), (jt == JT - 1)
                    nc.tensor.matmul(U_ps[:, 0:512], v_sl, E[:, 0:512],
                                     start=st, stop=sp)
                    nc.tensor.matmul(U_ps[:, 512:1024], v_sl, E[:, 512:1024],
                                     start=st, stop=sp)
                    col0 = (jt // 8) * 64 + (jt % 8)
                    for ib in range(8):
                        nc.tensor.matmul(
                            Z_ps[:, col0 + ib * 8:col0 + ib * 8 + 1],
                            E[:, ib * 128:(ib + 1) * 128], ones_t[:],
                            start=True, stop=True)

                zred = [None, None]

                def reduce_half(half):
                    zr = rz_pool.tile([128, 8], F32, tag=f"zred{half}", bufs=1,
                                      name="zred")
                    nc.vector.tensor_reduce(
                        zr[:],
                        Z_ps[:, half * 64:half * 64 + 64].rearrange(
                            "p (a b) -> p a b", a=8),
                        axis=AxX, op=Add)
                    zred[half] = zr

                score(0)
                score(1)
                for jt in range(JT):
                    av_z(jt)
                    if jt + 2 < JT:
                        score(jt + 2)
                    if jt == 8:
                        reduce_half(0)
                    if jt in PIECE_POINTS:
                        drain_at(jt)
                    if h == HC - 1 and u == 1 and jt in (3, 6, 9, 12, 14):
                        oproj_group(next(og_iter))

                # Evict U to SBUF on ACT right away so the U psum bank is
                # free before the next slot's first AV matmul.
                usb = rz_pool.tile([128, 1024], BF16, tag="usb", bufs=1,
                                   name="usb")
                nc.vector.tensor_copy(usb[:], U_ps[:])
                # softmax denominator: second-half reduce, combine, recip,
                # transpose to row form, partition-broadcast, normalize.
                reduce_half(1)
                zs = rz_pool.tile([128, 8], F32, tag="zs", bufs=1, name="zs")
                nc.vector.tensor_add(zs[:], zred[0][:], zred[1][:])
                rz = rz_pool.tile([128, 8], F32, tag="rz", bufs=1, name="rz")
                nc.vector.reciprocal(rz[:], zs[:])
                nc.tensor.matmul(Z_ps[0:8, 256:384], rz[:], ident_t[:],
                                 start=True, stop=True, is_transpose=True)
                rzt = rz_pool.tile([8, 128], BF16, tag="rzt", bufs=1, name="rzt")
                nc.vector.tensor_copy(rzt[:], Z_ps[0:8, 256:384])
                # [8,128] rows -> one [1,1024] partition-0 row (DMA, off the
                # critical path), then a single gpsimd broadcast to all
                # partitions for the normalize multiply.
                # flatten [8,128] -> [1,1024] via a DRAM round trip (plain
                # DMAs; engine-free and off the critical path - OT is only
                # consumed by the output projection much later).
                rsc = dram_pool.tile([8, 128], BF16, tag="rsc", bufs=2,
                                     name="rsc")
                nc.sync.dma_start(rsc[:], rzt[:])
                rzrow = rz_pool.tile([1, 1024], BF16, tag="rzrow", bufs=1,
                                     name="rzrow")
                nc.sync.dma_start(
                    rzrow[:], rsc.rearrange("a b -> (a b)").unsqueeze(0))
                rzf = rz_pool.tile([128, 1024], BF16, tag="rzf", bufs=1,
                                   name="rzf")
                nc.gpsimd.partition_broadcast(rzf[:], rzrow[0:1, :])
                nc.gpsimd.tensor_mul(OT[:, h, i0:i0 + 1024], usb[:], rzf[:])

            if h == HC - 2:
                # X no longer needed (last q-projection emitted); swap the
                # X heap space for the wo tiles.
                flush_filler()
                x_ctx.close()
                wo_pool = ctx.enter_context(
                    tc.tile_pool(name="wo", bufs=1, side="left"))
                for hh in range(HC):
                    wo_t = wo_pool.tile([128, S], BF16, tag=f"wo{hh}", bufs=1,
                                        name="wo_t")
                    nc.sync.dma_start(wo_t[:], wot_d[hh])
                    wo_ts.append(wo_t)
        while filler:
            pop_filler(1)

    pq_ctx.close()

    # ---------------- output projection ----------------
    with (
        tc.tile_pool(name="p_y", bufs=1, space="PSUM", side="right") as p_y,
        tc.tile_pool(name="ys", bufs=1, side="right") as ys_pool,
    ):
        ev = 0
        for tt in range(16):
            for ob in range(4):
                if (tt, ob) in prefilled:
                    continue
                o0 = ob * 512
                ps = p_y.tile([128, 512], F32, tag="y", bufs=8, name="ps_y")
                for h in range(HC):
                    nc.tensor.matmul(ps[:], OT[:, h, tt * 128:(tt + 1) * 128],
                                     wo_ts[h][:, o0:o0 + 512],
                                     start=(h == 0), stop=(h == HC - 1))
                yt = ys_pool.tile([128, 512], F32, tag="yt", bufs=8, name="yt")
                if ev % 2 == 0:
                    nc.scalar.copy(yt[:], ps[:])
                else:
                    nc.vector.tensor_copy(yt[:], ps[:])
                ev += 1
                nc.sync.dma_start(y_d[tt * 128:(tt + 1) * 128, o0:o0 + 512],
                                  yt[:])

    ctx.close()


def _build(repeat=1):
    import concourse.mybir as mybir
    import concourse.tile as tile
    from concourse import bacc

    F32 = mybir.dt.float32
    BF16 = mybir.dt.bfloat16

    nc = bacc.Bacc("TRN2", target_bir_lowering=False, debug=False)
    xt_d = nc.dram_tensor("xt", [128, CT, S], BF16, kind="ExternalInput").ap()
    cosk_d = nc.dram_tensor("cosk", [128, S], F32, kind="ExternalInput").ap()
    sinkm_d = nc.dram_tensor("sinkm", [128, S], F32, kind="ExternalInput").ap()
    wqt_d = nc.dram_tensor("wqt", [HC, 128, CT, 128], BF16,
                           kind="ExternalInput").ap()
    wkt_d = nc.dram_tensor("wkt", [KVC, 128, CT, 128], BF16,
                           kind="ExternalInput").ap()
    wvt_d = nc.dram_tensor("wvt", [128, CT, 256], BF16,
                           kind="ExternalInput").ap()
    wot_d = nc.dram_tensor("wot", [HC, 128, S], BF16, kind="ExternalInput").ap()
    ones_d = nc.dram_tensor("ones", [128, 1], BF16, kind="ExternalInput").ap()
    ident_d = nc.dram_tensor("ident", [128, 128], F32,
                             kind="ExternalInput").ap()
    y_d = nc.dram_tensor("y", [S, HID], F32, kind="ExternalOutput").ap()

    with tile.TileContext(nc) as tc:
        for _ in range(repeat):
            _emit(nc, tc, (xt_d, cosk_d, sinkm_d, wqt_d, wkt_d, wvt_d, wot_d,
                           ones_d, ident_d, y_d))
    nc.compile()
    return nc


class _Runner:
    """Persistent-jit PJRT executor (axon) / NRT executor (native)."""

    def __init__(self, nc):
        self.nc = nc
        from concourse._compat import axon_active
        self.axon = axon_active()
        if not self.axon:
            return
        import jax
        from jax.sharding import Mesh, PartitionSpec
        from jax.experimental.shard_map import shard_map
        import concourse.mybir as mybir
        from concourse.bass2jax import (
            _bass_exec_p, install_neuronx_cc_hook, partition_id_tensor)

        install_neuronx_cc_hook()
        partition_name = (nc.partition_id_tensor.name
                          if nc.partition_id_tensor else None)
        in_names, out_names, out_avals, zero_outs = [], [], [], []
        for alloc in nc.m.functions[0].allocations:
            if not isinstance(alloc, mybir.MemoryLocationSet):
                continue
            name = alloc.memorylocations[0].name
            if alloc.kind == "ExternalInput":
                if name != partition_name:
                    in_names.append(name)
            elif alloc.kind == "ExternalOutput":
                shape = tuple(alloc.tensor_shape)
                dtype = mybir.dt.np(alloc.dtype)
                out_names.append(name)
                out_avals.append(jax.core.ShapedArray(shape, dtype))
                zero_outs.append(np.zeros(shape, dtype))
        self.in_names, self.out_names = in_names, out_names
        self.zero_outs = zero_outs
        n_params, n_outs = len(in_names), len(out_names)
        all_in = in_names + out_names
        if partition_name is not None:
            all_in.append(partition_name)
        donate = tuple(range(n_params, n_params + n_outs))

        def _body(*args):
            operands = list(args)
            if partition_name is not None:
                operands.append(partition_id_tensor())
            return tuple(_bass_exec_p.bind(
                *operands,
                out_avals=tuple(out_avals),
                in_names=tuple(all_in),
                out_names=tuple(out_names),
                lowering_input_output_aliases=(),
                sim_require_finite=True,
                sim_require_nnan=True,
                nc=nc,
            ))

        devices = jax.devices()[:N_CORES]
        mesh = Mesh(np.asarray(devices), ("core",))
        self._fn = jax.jit(
            shard_map(_body, mesh=mesh,
                      in_specs=(PartitionSpec("core"),) * (n_params + n_outs),
                      out_specs=(PartitionSpec("core"),) * n_outs,
                      check_rep=False),
            donate_argnums=donate, keep_unused=True,
        )

    def run(self, in_maps):
        if not self.axon:
            from concourse import bass_utils
            res = bass_utils.run_bass_kernel_spmd(
                self.nc, in_maps, core_ids=list(range(N_CORES)))
            return res.results
        concat_in = [
            np.concatenate([np.asarray(in_maps[c][n]) for c in range(N_CORES)],
                           axis=0)
            for n in self.in_names
        ] + [np.concatenate([z] * N_CORES, axis=0) for z in self.zero_outs]
        outs = [np.asarray(o) for o in self._fn(*concat_in)]
        per_core = []
        for c in range(N_CORES):
            d = {}
            for name, o in zip(self.out_names, outs):
                rows = o.shape[0] // N_CORES
                d[name] = o[c * rows:(c + 1) * rows]
            per_core.append(d)
        return per_core


def _prep_inputs(x, cos, sin, wq, wk, wv, wo):
    import concourse.mybir as mybir
    f32 = np.float32
    bf16 = mybir.dt.np(mybir.dt.bfloat16)

    cosT = np.ascontiguousarray(np.asarray(cos).T.astype(f32))    # [128, S]
    sinm = np.ascontiguousarray(np.asarray(sin).T.astype(f32))
    sinm[0:64] *= -1.0
    ones = np.ones((128, 1), bf16)
    ident = np.eye(128, dtype=f32)
    x = np.asarray(x, f32)
    wq = np.asarray(wq, f32)
    wk = np.asarray(wk, f32)
    wv = np.asarray(wv, f32)
    wo = np.asarray(wo, f32)

    in_maps = []
    for c in range(N_CORES):
        b, kh = c // 2, c % 2
        # X^T packed [p, ct, tok]: [p, ct, j] = x[b, j, ct*128+p]
        xt = np.ascontiguousarray(
            x[b].T.reshape(CT, 128, S).transpose(1, 0, 2)).astype(bf16)
        # wq rows for this core's heads -> [h, p(ct-part), ct, c(col)]
        wq_c = wq[kh * 1024:(kh + 1) * 1024, :]
        wqt = np.ascontiguousarray(
            wq_c.reshape(HC, 128, CT, 128).transpose(0, 3, 2, 1)).astype(bf16)
        wk_c = wk[kh * 256:(kh + 1) * 256, :]
        wkt = np.ascontiguousarray(
            wk_c.reshape(KVC, 128, CT, 128).transpose(0, 3, 2, 1)).astype(bf16)
        wv_c = wv[kh * 256:(kh + 1) * 256, :]
        wvt = np.ascontiguousarray(
            wv_c.reshape(256, CT, 128).transpose(2, 1, 0)).astype(bf16)
        # wo columns for this core's heads -> [h, p(=d), out]
        wot = np.ascontiguousarray(
            wo[:, kh * 1024:(kh + 1) * 1024].T.reshape(HC, 128, S)).astype(bf16)
        in_maps.append({
            "xt": xt, "cosk": cosT, "sinkm": sinm,
            "wqt": wqt, "wkt": wkt, "wvt": wvt, "wot": wot,
            "ones": ones, "ident": ident,
        })
    return in_maps


def kernel(x, cos, sin, wq, wk, wv, wo):
    if "nc" not in _cache:
        _cache["nc"] = _build()
        _cache["runner"] = _Runner(_cache["nc"])
    runner = _cache["runner"]
    in_maps = _prep_inputs(x, cos, sin, wq, wk, wv, wo)
    results = runner.run(in_maps)
    y = np.empty((B, S, HID), np.float32)
    for b in range(B):
        y[b] = (results[2 * b]["y"].astype(np.float32)
                + results[2 * b + 1]["y"].astype(np.float32))
    return y


# revision 4
# speedup vs baseline: 1.0005x; 1.0005x over previous
"""Trainium2 Bass kernel for nn_Attention_55130200211640 (v2).

GQA attention block: q/k/v projections + RoPE (theta=1e6) + non-causal
softmax attention (16 q-heads, 4 kv-heads, head_dim 128) + output
projection. B=4, S=2048, HID=2048, fp32 I/O.

Sharding: (batch x 4) x (kv-group-half x 2) = 8 cores, tensor-parallel
over heads within a batch. Each core computes, for its batch, the full
2048-token sequence restricted to 2 of the 4 kv groups (= 8 of the 16
q heads): q/k/v projections, RoPE, attention, and a PARTIAL o_proj
(contraction over its 8 heads only). The host adds the two partial y's
per batch (all-reduce done on host; sim time measures one core's NEFF).

Everything computes in bf16 on the PE (psum f32), which both halves
SBUF/DMA footprint (no DRAM bounce at all: X^T stays resident in SBUF
through the whole kernel) and keeps 1 cycle/row matmul throughput.

Per-core dataflow ("contraction-on-partition" layouts everywhere):
  preamble: X^T [hid,2048] bf16 resident; K^T [d,2|S] and V [j,256]
            computed+roped, Q^T head 0.
  slots:    16 slots = (8 heads) x (2 query halves of 1024). Per slot:
            S^T[j,i] = K^T_g . Q^T_h on PE -> exp on ACT (scale folded)
            -> E bf16; U^T[d,i] = sum_j V E (psum-accumulated);
            Z[i] = sum_j E via 128 stationary-E matmuls with a [128,1]
            ones moving operand (1 row each - partition reduction at
            free-size cost); Z cols -> free-axis reduce (DVE) -> recip
            -> PE transpose -> per-block gpsimd partition_broadcast ->
            one DVE mul U*(1/Z) -> OT bf16. Next head's q-projection
            matmuls + rope are interleaved into the slot as PE filler
            so the PE never idles while ACT paces the exps.
  o_proj:   y[i,o] = sum_h OT_h . wo_h, psum-accumulated over the 8
            heads, 8-bank rotation, evicted via ACT/DVE/Pool copies.
"""

import numpy as np

B, S, HID = 4, 2048, 2048
H, KV, D = 16, 4, 128
N_CORES = 8
HC = 8                 # heads per core
KVC = 2                # kv groups per core
CT = HID // 128        # contraction tiles
JT = S // 128          # key tiles
SCALE = 1.0 / float(np.sqrt(D))

_cache = {}


def _emit(nc, tc, io):
    import concourse.mybir as mybir
    from collections import deque
    from contextlib import ExitStack

    F32 = mybir.dt.float32
    BF16 = mybir.dt.bfloat16
    Exp = mybir.ActivationFunctionType.Exp
    AxX = mybir.AxisListType.X
    Add = mybir.AluOpType.add

    xt_d, cosk_d, sinkm_d, wqt_d, wkt_d, wvt_d, wot_d, ones_d, ident_d, y_d = io

    ctx = ExitStack()

    # ---------------- persistent SBUF tiles (left heap) ----------------
    const_pool = ctx.enter_context(tc.tile_pool(name="const", bufs=1, side="left"))
    ones_t = const_pool.tile([128, 1], BF16)
    ident_t = const_pool.tile([128, 128], F32)
    COS = const_pool.tile([128, S], BF16)
    SINM = const_pool.tile([128, S], BF16)

    dram_pool = ctx.enter_context(tc.tile_pool(name="drp", bufs=1, space="DRAM"))
    kv_pool = ctx.enter_context(tc.tile_pool(name="kv", bufs=1, side="left"))
    KT = kv_pool.tile([128, KVC, S], BF16)        # [d, g, j]
    VV = kv_pool.tile([128, JT, KVC * 128], BF16)  # [j, jt, g*128+d]
    q_pool = ctx.enter_context(tc.tile_pool(name="qt", bufs=1, side="left"))
    QT = q_pool.tile([128, HC, S], BF16)          # [d, h, i]
    o_pool = ctx.enter_context(tc.tile_pool(name="ot", bufs=1, side="left"))
    OT = o_pool.tile([128, HC, S], BF16)          # [d, h, i]

    # X^T resident for the whole projection span. Opened last on the left
    # heap so it can be closed (LIFO) mid-emission to make room for wo.
    x_ctx = ExitStack()
    x_pool = x_ctx.enter_context(tc.tile_pool(name="xp", bufs=1, side="left"))
    X = x_pool.tile([128, CT, S], BF16)           # [hid%128, ct, tok]

    w_pool = ctx.enter_context(tc.tile_pool(name="wp", bufs=2, side="right"))
    st_pool = ctx.enter_context(tc.tile_pool(name="st", bufs=1, side="right"))
    e_pool = ctx.enter_context(tc.tile_pool(name="ep", bufs=3, side="right"))
    rz_pool = ctx.enter_context(tc.tile_pool(name="rz", bufs=1, side="right"))

    # Persistent psum pool for projection chunks: 1 bank, used by the
    # q-projection filler during the slots as well.
    pq_ctx = ExitStack()
    p_q = pq_ctx.enter_context(
        tc.tile_pool(name="p_q", bufs=1, space="PSUM", side="left"))

    def rope(ps, c0, n, dst):
        """RoPE a [128, n] psum tile (layout [d, pos], positions c0:c0+n)
        -> bf16 SBUF dst. rotate_half is a cross-partition half-swap; the
        sign lives in SINM (rows 0:64 pre-negated on the host)."""
        tmp = st_pool.tile([128, 512], F32, tag="tmp", bufs=2, name="tmp")
        stage = st_pool.tile([128, 512], F32, tag="stage", bufs=2, name="stage")
        nc.vector.tensor_mul(stage[0:64, 0:n], ps[64:128, :],
                             SINM[0:64, c0:c0 + n])
        nc.vector.tensor_mul(stage[64:128, 0:n], ps[0:64, :],
                             SINM[64:128, c0:c0 + n])
        nc.vector.tensor_mul(tmp[:, 0:n], ps[:], COS[:, c0:c0 + n])
        nc.vector.tensor_add(dst, stage[:, 0:n], tmp[:, 0:n])

    # ---------------- preamble: K, V, Q(0) projections ----------------
    # DMA priority: feed the PE within ~4.5us and keep the X stream just
    # ahead of the 256-token preamble chunk walk (6.8us compute / 2.9us
    # DMA per chunk).
    wk_ts = []
    wk_t0 = w_pool.tile([128, CT, 128], BF16, tag="w", bufs=3, name="wk_t")
    nc.sync.dma_start(wk_t0[:], wkt_d[0])
    wk_ts.append(wk_t0)
    nc.sync.dma_start(X[:, :, 0:256], xt_d[:, :, 0:256])
    nc.scalar.dma_start(COS[:, 0:512], cosk_d[:, 0:512])
    nc.scalar.dma_start(SINM[:, 0:512], sinkm_d[:, 0:512])
    wk_t1 = w_pool.tile([128, CT, 128], BF16, tag="w", bufs=3, name="wk_t")
    nc.sync.dma_start(wk_t1[:], wkt_d[1])
    wk_ts.append(wk_t1)
    wv_t = w_pool.tile([128, CT, 256], BF16, tag="wv", bufs=1, name="wv_t")
    nc.sync.dma_start(wv_t[:], wvt_d[:])
    nc.sync.dma_start(X[:, :, 256:512], xt_d[:, :, 256:512])
    nc.scalar.dma_start(COS[:, 512:1024], cosk_d[:, 512:1024])
    nc.scalar.dma_start(SINM[:, 512:1024], sinkm_d[:, 512:1024])
    nc.sync.dma_start(X[:, :, 512:768], xt_d[:, :, 512:768])
    nc.scalar.dma_start(COS[:, 1024:2048], cosk_d[:, 1024:2048])
    nc.scalar.dma_start(SINM[:, 1024:2048], sinkm_d[:, 1024:2048])
    for tch in range(3, 8):
        t0 = tch * 256
        nc.sync.dma_start(X[:, :, t0:t0 + 256], xt_d[:, :, t0:t0 + 256])
    wq_t0 = w_pool.tile([128, CT, 128], BF16, tag="w", bufs=3, name="wq_t")
    nc.sync.dma_start(wq_t0[:], wqt_d[0])
    nc.sync.dma_start(ones_t[:], ones_d[:])
    nc.sync.dma_start(ident_t[:], ident_d[:])

    with tc.tile_pool(name="p_pre", bufs=1, space="PSUM", side="right") as p_pre:
        for tch in range(8):
            j0 = tch * 256
            for g in range(KVC):
                ps = p_pre.tile([128, 512], F32, tag="c", bufs=6, name="ps_pre")
                for ct in range(CT):
                    nc.tensor.matmul(ps[:, 0:256], wk_ts[g][:, ct, :],
                                     X[:, ct, j0:j0 + 256],
                                     start=(ct == 0), stop=(ct == CT - 1))
                rope(ps[:, 0:256], j0, 256, KT[:, g, j0:j0 + 256])
            for jt in range(tch * 2, tch * 2 + 2):
                ps = p_pre.tile([128, 512], F32, tag="c", bufs=6, name="ps_pre")
                for ct in range(CT):
                    nc.tensor.matmul(ps[:, 0:256],
                                     X[:, ct, jt * 128:(jt + 1) * 128],
                                     wv_t[:, ct, :],
                                     start=(ct == 0), stop=(ct == CT - 1))
                nc.scalar.copy(VV[:, jt, :], ps[:, 0:256])
        # Q projection head 0.
        for qc in range(4):
            i0 = qc * 512
            ps = p_pre.tile([128, 512], F32, tag="c", bufs=6, name="ps_pre")
            for ct in range(CT):
                nc.tensor.matmul(ps[:], wq_t0[:, ct, :],
                                 X[:, ct, i0:i0 + 512],
                                 start=(ct == 0), stop=(ct == CT - 1))
            rope(ps, i0, 512, QT[:, 0, i0:i0 + 512])

    # ---------------- q-projection filler machinery ----------------
    filler = deque()

    wq_pref = {}

    def prefetch_wq(h):
        if h < HC and h not in wq_pref:
            wq_t = w_pool.tile([128, CT, 128], BF16, tag="w", bufs=3,
                               name="wq_t")
            nc.sync.dma_start(wq_t[:], wqt_d[h])
            wq_pref[h] = wq_t

    def push_qproj(h):
        """Queue head h's q-projection as small PE filler pieces."""
        prefetch_wq(h)
        state = {"w": wq_pref.pop(h)}

        for qc in range(4):
            def open_chunk(qc=qc):
                ps = p_q.tile([128, 512], F32, tag="q", bufs=1, name="ps_q")
                state["ps"] = ps
            filler.append((False, open_chunk))
            for c4 in range(4):
                def mm_piece(qc=qc, c4=c4):
                    ps = state["ps"]
                    for ct in range(c4 * 4, c4 * 4 + 4):
                        nc.tensor.matmul(
                            ps[:], state["w"][:, ct, :],
                            X[:, ct, qc * 512:qc * 512 + 512],
                            start=(ct == 0), stop=(ct == CT - 1))
                filler.append((True, mm_piece))

            def rope_piece(qc=qc):
                rope(state["ps"], qc * 512, 512, QT[:, h, qc * 512:qc * 512 + 512])
            filler.append((False, rope_piece))

    # Fixed-position pacing: a qproj chunk is 6 pieces (open, 4 mm, rope)
    # emitted at jts {0..4, 8} and {8..12, ...} of each slot, i.e. two
    # chunks per slot, finishing well clear of the slot boundary so the
    # single qproj psum bank and the DVE rope never collide with the
    # slot-end usb/normalize chain.
    PIECE_POINTS = (0, 1, 2, 3, 4, 5, 8, 9, 10, 11, 12, 13)

    def drain_at(point):
        k = 0
        while filler and k < PIECE_POINTS.count(point):
            filler.popleft()[1]()
            k += 1

    def flush_filler():
        while filler:
            filler.popleft()[1]()

    # ---------------- attention slots ----------------
    wo_ts = []
    prefilled = set()
    ys_ev = [0]

    def oproj_group(key):
        tt, ob = key
        o0 = ob * 512
        ps = p_q.tile([128, 512], F32, tag="q", bufs=1, name="ps_q")
        for hh in range(HC):
            nc.tensor.matmul(ps[:], OT[:, hh, tt * 128:(tt + 1) * 128],
                             wo_ts[hh][:, o0:o0 + 512],
                             start=(hh == 0), stop=(hh == HC - 1))
        yt = st_pool.tile([128, 512], F32, tag="yt8", bufs=2, name="yt")
        if ys_ev[0] % 2 == 0:
            nc.scalar.copy(yt[:], ps[:])
        else:
            nc.vector.tensor_copy(yt[:], ps[:])
        ys_ev[0] += 1
        nc.sync.dma_start(y_d[tt * 128:(tt + 1) * 128, o0:o0 + 512], yt[:])
        prefilled.add(key)

    og_iter = iter([(0, 0), (0, 1), (0, 2), (0, 3), (1, 0)])
    with (
        tc.tile_pool(name="p_s", bufs=1, space="PSUM", side="right") as p_s,
        tc.tile_pool(name="p_u", bufs=1, space="PSUM", side="right") as p_u,
        tc.tile_pool(name="p_z", bufs=1, space="PSUM", side="right") as p_z,
    ):
        for h in range(HC):
            g = h // 4
            prefetch_wq(h + 2)
            if h + 1 < HC:
                push_qproj(h + 1)
            for u in range(2):
                i0 = u * 1024
                U_ps = p_u.tile([128, 1024], F32, tag="U", bufs=1, name="ps_U")
                Z_ps = p_z.tile([128, 512], F32, tag="Z", bufs=1, name="ps_Z")
                Es = {}

                def score(jt):
                    ps = p_s.tile([128, 1024], F32, tag="S", bufs=2, name="ps_S")
                    kt_sl = KT[:, g, jt * 128:(jt + 1) * 128]
                    nc.tensor.matmul(ps[:, 0:512], kt_sl,
                                     QT[:, h, i0:i0 + 512],
                                     start=True, stop=True)
                    nc.tensor.matmul(ps[:, 512:1024], kt_sl,
                                     QT[:, h, i0 + 512:i0 + 1024],
                                     start=True, stop=True)
                    E = e_pool.tile([128, 1024], BF16, tag="e", bufs=4, name="E")
                    nc.scalar.activation(E[:], ps[:], Exp, scale=SCALE)
                    Es[jt] = E

                def av_z(jt):
                    # Z column layout: [jt-half, ib, jt%8] so the first
                    # half's reduce can run mid-slot and the next slot's
                    # writes never collide with this slot's late reduce.
                    E = Es.pop(jt)
                    v_sl = VV[:, jt, g * 128:(g + 1) * 128]
                    st, sp = (jt == 0), (jt == JT - 1)
                    nc.tensor.matmul(U_ps[:, 0:512], v_sl, E[:, 0:512],
                                     start=st, stop=sp)
                    nc.tensor.matmul(U_ps[:, 512:1024], v_sl, E[:, 512:1024],
                                     start=st, stop=sp)
                    col0 = (jt // 8) * 64 + (jt % 8)
                    for ib in range(8):
                        nc.tensor.matmul(
                            Z_ps[:, col0 + ib * 8:col0 + ib * 8 + 1],
                            E[:, ib * 128:(ib + 1) * 128], ones_t[:],
                            start=True, stop=True)

                zred = [None, None]

                def reduce_half(half):
                    zr = rz_pool.tile([128, 8], F32, tag=f"zred{half}", bufs=1,
                                      name="zred")
                    nc.vector.tensor_reduce(
                        zr[:],
                        Z_ps[:, half * 64:half * 64 + 64].rearrange(
                            "p (a b) -> p a b", a=8),
                        axis=AxX, op=Add)
                    zred[half] = zr

                score(0)
                score(1)
                for jt in range(JT):
                    av_z(jt)
                    if jt + 2 < JT:
                        score(jt + 2)
                    if jt == 8:
                        reduce_half(0)
                    if jt in PIECE_POINTS:
                        drain_at(jt)
                    if h == HC - 1 and u == 1 and jt in (3, 6, 9, 12, 14):
                        oproj_group(next(og_iter))

                # Evict U to SBUF on ACT right away so the U psum bank is
                # free before the next slot's first AV matmul.
                usb = rz_pool.tile([128, 1024], BF16, tag="usb", bufs=1,
                                   name="usb")
                nc.vector.tensor_copy(usb[:], U_ps[:])
                # softmax denominator: second-half reduce, combine, recip,
                # transpose to row form, partition-broadcast, normalize.
                reduce_half(1)
                zs = rz_pool.tile([128, 8], F32, tag="zs", bufs=1, name="zs")
                nc.vector.tensor_add(zs[:], zred[0][:], zred[1][:])
                rz = rz_pool.tile([128, 8], F32, tag="rz", bufs=1, name="rz")
                nc.vector.reciprocal(rz[:], zs[:])
                nc.tensor.matmul(Z_ps[0:8, 256:384], rz[:], ident_t[:],
                                 start=True, stop=True, is_transpose=True)
                rzt = rz_pool.tile([8, 128], BF16, tag="rzt", bufs=1, name="rzt")
                nc.vector.tensor_copy(rzt[:], Z_ps[0:8, 256:384])
                # [8,128] rows -> one [1,1024] partition-0 row (DMA, off the
                # critical path), then a single gpsimd broadcast to all
                # partitions for the normalize multiply.
                # flatten [8,128] -> [1,1024] via a DRAM round trip (plain
                # DMAs; engine-free and off the critical path - OT is only
                # consumed by the output projection much later).
                rsc = dram_pool.tile([8, 128], BF16, tag="rsc", bufs=2,
                                     name="rsc")
                nc.sync.dma_start(rsc[:], rzt[:])
                rzrow = rz_pool.tile([1, 1024], BF16, tag="rzrow", bufs=1,
                                     name="rzrow")
                nc.sync.dma_start(
                    rzrow[:], rsc.rearrange("a b -> (a b)").unsqueeze(0))
                rzf = rz_pool.tile([128, 1024], BF16, tag="rzf", bufs=1,
                                   name="rzf")
                nc.gpsimd.partition_broadcast(rzf[:], rzrow[0:1, :])
                nc.gpsimd.tensor_mul(OT[:, h, i0:i0 + 1024], usb[:], rzf[:])

            if h == HC - 2:
                # X no longer needed (last q-projection emitted); swap the
                # X heap space for the wo tiles.
                flush_filler()
                x_ctx.close()
                wo_pool = ctx.enter_context(
                    tc.tile_pool(name="wo", bufs=1, side="left"))
                for hh in range(HC):
                    wo_t = wo_pool.tile([128, S], BF16, tag=f"wo{hh}", bufs=1,
                                        name="wo_t")
                    nc.sync.dma_start(wo_t[:], wot_d[hh])
                    wo_ts.append(wo_t)
        while filler:
            pop_filler(1)

    pq_ctx.close()

    # ---------------- output projection ----------------
    with (
        tc.tile_pool(name="p_y", bufs=1, space="PSUM", side="right") as p_y,
        tc.tile_pool(name="ys", bufs=1, side="right") as ys_pool,
    ):
        ev = 0
        for tt in range(16):
            for ob in range(4):
                if (tt, ob) in prefilled:
                    continue
                o0 = ob * 512
                ps = p_y.tile([128, 512], F32, tag="y", bufs=8, name="ps_y")
                for h in range(HC):
                    nc.tensor.matmul(ps[:], OT[:, h, tt * 128:(tt + 1) * 128],
                                     wo_ts[h][:, o0:o0 + 512],
                                     start=(h == 0), stop=(h == HC - 1))
                yt = ys_pool.tile([128, 512], F32, tag="yt", bufs=8, name="yt")
                if ev % 2 == 0:
                    nc.scalar.copy(yt[:], ps[:])
                else:
                    nc.vector.tensor_copy(yt[:], ps[:])
                ev += 1
                nc.sync.dma_start(y_d[tt * 128:(tt + 1) * 128, o0:o0 + 512],
                                  yt[:])

    ctx.close()


def _build(repeat=1):
    import concourse.mybir as mybir
    import concourse.tile as tile
    from concourse import bacc

    F32 = mybir.dt.float32
    BF16 = mybir.dt.bfloat16

    nc = bacc.Bacc("TRN2", target_bir_lowering=False, debug=False)
    xt_d = nc.dram_tensor("xt", [128, CT, S], BF16, kind="ExternalInput").ap()
    cosk_d = nc.dram_tensor("cosk", [128, S], BF16, kind="ExternalInput").ap()
    sinkm_d = nc.dram_tensor("sinkm", [128, S], BF16,
                             kind="ExternalInput").ap()
    wqt_d = nc.dram_tensor("wqt", [HC, 128, CT, 128], BF16,
                           kind="ExternalInput").ap()
    wkt_d = nc.dram_tensor("wkt", [KVC, 128, CT, 128], BF16,
                           kind="ExternalInput").ap()
    wvt_d = nc.dram_tensor("wvt", [128, CT, 256], BF16,
                           kind="ExternalInput").ap()
    wot_d = nc.dram_tensor("wot", [HC, 128, S], BF16, kind="ExternalInput").ap()
    ones_d = nc.dram_tensor("ones", [128, 1], BF16, kind="ExternalInput").ap()
    ident_d = nc.dram_tensor("ident", [128, 128], F32,
                             kind="ExternalInput").ap()
    y_d = nc.dram_tensor("y", [S, HID], F32, kind="ExternalOutput").ap()

    with tile.TileContext(nc) as tc:
        for _ in range(repeat):
            _emit(nc, tc, (xt_d, cosk_d, sinkm_d, wqt_d, wkt_d, wvt_d, wot_d,
                           ones_d, ident_d, y_d))
    nc.compile()
    return nc


class _Runner:
    """Persistent-jit PJRT executor (axon) / NRT executor (native)."""

    def __init__(self, nc):
        self.nc = nc
        from concourse._compat import axon_active
        self.axon = axon_active()
        if not self.axon:
            return
        import jax
        from jax.sharding import Mesh, PartitionSpec
        from jax.experimental.shard_map import shard_map
        import concourse.mybir as mybir
        from concourse.bass2jax import (
            _bass_exec_p, install_neuronx_cc_hook, partition_id_tensor)

        install_neuronx_cc_hook()
        partition_name = (nc.partition_id_tensor.name
                          if nc.partition_id_tensor else None)
        in_names, out_names, out_avals, zero_outs = [], [], [], []
        for alloc in nc.m.functions[0].allocations:
            if not isinstance(alloc, mybir.MemoryLocationSet):
                continue
            name = alloc.memorylocations[0].name
            if alloc.kind == "ExternalInput":
                if name != partition_name:
                    in_names.append(name)
            elif alloc.kind == "ExternalOutput":
                shape = tuple(alloc.tensor_shape)
                dtype = mybir.dt.np(alloc.dtype)
                out_names.append(name)
                out_avals.append(jax.core.ShapedArray(shape, dtype))
                zero_outs.append(np.zeros(shape, dtype))
        self.in_names, self.out_names = in_names, out_names
        self.zero_outs = zero_outs
        n_params, n_outs = len(in_names), len(out_names)
        all_in = in_names + out_names
        if partition_name is not None:
            all_in.append(partition_name)
        donate = tuple(range(n_params, n_params + n_outs))

        def _body(*args):
            operands = list(args)
            if partition_name is not None:
                operands.append(partition_id_tensor())
            return tuple(_bass_exec_p.bind(
                *operands,
                out_avals=tuple(out_avals),
                in_names=tuple(all_in),
                out_names=tuple(out_names),
                lowering_input_output_aliases=(),
                sim_require_finite=True,
                sim_require_nnan=True,
                nc=nc,
            ))

        devices = jax.devices()[:N_CORES]
        mesh = Mesh(np.asarray(devices), ("core",))
        self._fn = jax.jit(
            shard_map(_body, mesh=mesh,
                      in_specs=(PartitionSpec("core"),) * (n_params + n_outs),
                      out_specs=(PartitionSpec("core"),) * n_outs,
                      check_rep=False),
            donate_argnums=donate, keep_unused=True,
        )

    def run(self, in_maps):
        if not self.axon:
            from concourse import bass_utils
            res = bass_utils.run_bass_kernel_spmd(
                self.nc, in_maps, core_ids=list(range(N_CORES)))
            return res.results
        concat_in = [
            np.concatenate([np.asarray(in_maps[c][n]) for c in range(N_CORES)],
                           axis=0)
            for n in self.in_names
        ] + [np.concatenate([z] * N_CORES, axis=0) for z in self.zero_outs]
        outs = [np.asarray(o) for o in self._fn(*concat_in)]
        per_core = []
        for c in range(N_CORES):
            d = {}
            for name, o in zip(self.out_names, outs):
                rows = o.shape[0] // N_CORES
                d[name] = o[c * rows:(c + 1) * rows]
            per_core.append(d)
        return per_core


def _prep_inputs(x, cos, sin, wq, wk, wv, wo):
    import concourse.mybir as mybir
    f32 = np.float32
    bf16 = mybir.dt.np(mybir.dt.bfloat16)

    cosT = np.asarray(cos).T.astype(f32)    # [128, S]
    sinm = np.asarray(sin).T.astype(f32).copy()
    sinm[0:64] *= -1.0
    cosT = np.ascontiguousarray(cosT).astype(bf16)
    sinm = np.ascontiguousarray(sinm).astype(bf16)
    ones = np.ones((128, 1), bf16)
    ident = np.eye(128, dtype=f32)
    x = np.asarray(x, f32)
    wq = np.asarray(wq, f32)
    wk = np.asarray(wk, f32)
    wv = np.asarray(wv, f32)
    wo = np.asarray(wo, f32)

    in_maps = []
    for c in range(N_CORES):
        b, kh = c // 2, c % 2
        # X^T packed [p, ct, tok]: [p, ct, j] = x[b, j, ct*128+p]
        xt = np.ascontiguousarray(
            x[b].T.reshape(CT, 128, S).transpose(1, 0, 2)).astype(bf16)
        # wq rows for this core's heads -> [h, p(ct-part), ct, c(col)]
        wq_c = wq[kh * 1024:(kh + 1) * 1024, :]
        wqt = np.ascontiguousarray(
            wq_c.reshape(HC, 128, CT, 128).transpose(0, 3, 2, 1)).astype(bf16)
        wk_c = wk[kh * 256:(kh + 1) * 256, :]
        wkt = np.ascontiguousarray(
            wk_c.reshape(KVC, 128, CT, 128).transpose(0, 3, 2, 1)).astype(bf16)
        wv_c = wv[kh * 256:(kh + 1) * 256, :]
        wvt = np.ascontiguousarray(
            wv_c.reshape(256, CT, 128).transpose(2, 1, 0)).astype(bf16)
        # wo columns for this core's heads -> [h, p(=d), out]
        wot = np.ascontiguousarray(
            wo[:, kh * 1024:(kh + 1) * 1024].T.reshape(HC, 128, S)).astype(bf16)
        in_maps.append({
            "xt": xt, "cosk": cosT, "sinkm": sinm,
            "wqt": wqt, "wkt": wkt, "wvt": wvt, "wot": wot,
            "ones": ones, "ident": ident,
        })
    return in_maps


def kernel(x, cos, sin, wq, wk, wv, wo):
    if "nc" not in _cache:
        _cache["nc"] = _build()
        _cache["runner"] = _Runner(_cache["nc"])
    runner = _cache["runner"]
    in_maps = _prep_inputs(x, cos, sin, wq, wk, wv, wo)
    results = runner.run(in_maps)
    y = np.empty((B, S, HID), np.float32)
    for b in range(B):
        y[b] = (results[2 * b]["y"].astype(np.float32)
                + results[2 * b + 1]["y"].astype(np.float32))
    return y


# revision 5
# speedup vs baseline: 1.0027x; 1.0022x over previous
"""Trainium2 Bass kernel for nn_Attention_55130200211640 (v2).

GQA attention block: q/k/v projections + RoPE (theta=1e6) + non-causal
softmax attention (16 q-heads, 4 kv-heads, head_dim 128) + output
projection. B=4, S=2048, HID=2048, fp32 I/O.

Sharding: (batch x 4) x (kv-group-half x 2) = 8 cores, tensor-parallel
over heads within a batch. Each core computes, for its batch, the full
2048-token sequence restricted to 2 of the 4 kv groups (= 8 of the 16
q heads): q/k/v projections, RoPE, attention, and a PARTIAL o_proj
(contraction over its 8 heads only). The host adds the two partial y's
per batch (all-reduce done on host; sim time measures one core's NEFF).

Everything computes in bf16 on the PE (psum f32), which both halves
SBUF/DMA footprint (no DRAM bounce at all: X^T stays resident in SBUF
through the whole kernel) and keeps 1 cycle/row matmul throughput.

Per-core dataflow ("contraction-on-partition" layouts everywhere):
  preamble: X^T [hid,2048] bf16 resident; K^T [d,2|S] and V [j,256]
            computed+roped, Q^T head 0.
  slots:    16 slots = (8 heads) x (2 query halves of 1024). Per slot:
            S^T[j,i] = K^T_g . Q^T_h on PE -> exp on ACT (scale folded)
            -> E bf16; U^T[d,i] = sum_j V E (psum-accumulated);
            Z[i] = sum_j E via 128 stationary-E matmuls with a [128,1]
            ones moving operand (1 row each - partition reduction at
            free-size cost); Z cols -> free-axis reduce (DVE) -> recip
            -> PE transpose -> per-block gpsimd partition_broadcast ->
            one DVE mul U*(1/Z) -> OT bf16. Next head's q-projection
            matmuls + rope are interleaved into the slot as PE filler
            so the PE never idles while ACT paces the exps.
  o_proj:   y[i,o] = sum_h OT_h . wo_h, psum-accumulated over the 8
            heads, 8-bank rotation, evicted via ACT/DVE/Pool copies.
"""

import numpy as np

B, S, HID = 4, 2048, 2048
H, KV, D = 16, 4, 128
N_CORES = 8
HC = 8                 # heads per core
KVC = 2                # kv groups per core
CT = HID // 128        # contraction tiles
JT = S // 128          # key tiles
SCALE = 1.0 / float(np.sqrt(D))

_cache = {}


def _emit(nc, tc, io):
    import concourse.mybir as mybir
    from collections import deque
    from contextlib import ExitStack

    F32 = mybir.dt.float32
    BF16 = mybir.dt.bfloat16
    Exp = mybir.ActivationFunctionType.Exp
    AxX = mybir.AxisListType.X
    Add = mybir.AluOpType.add

    xt_d, cosk_d, sinkm_d, wqt_d, wkt_d, wvt_d, wot_d, ones_d, ident_d, y_d = io

    ctx = ExitStack()

    # ---------------- persistent SBUF tiles (left heap) ----------------
    const_pool = ctx.enter_context(tc.tile_pool(name="const", bufs=1, side="left"))
    ones_t = const_pool.tile([128, 1], BF16)
    ident_t = const_pool.tile([128, 128], F32)
    COS = const_pool.tile([128, S], BF16)
    SINM = const_pool.tile([128, S], BF16)

    dram_pool = ctx.enter_context(tc.tile_pool(name="drp", bufs=1, space="DRAM"))
    kv_pool = ctx.enter_context(tc.tile_pool(name="kv", bufs=1, side="left"))
    KT = kv_pool.tile([128, KVC, S], BF16)        # [d, g, j]
    VV = kv_pool.tile([128, JT, KVC * 128], BF16)  # [j, jt, g*128+d]
    q_pool = ctx.enter_context(tc.tile_pool(name="qt", bufs=1, side="left"))
    QT = q_pool.tile([128, HC, S], BF16)          # [d, h, i]
    o_pool = ctx.enter_context(tc.tile_pool(name="ot", bufs=1, side="left"))
    OT = o_pool.tile([128, HC, S], BF16)          # [d, h, i]

    # X^T resident for the whole projection span. Opened last on the left
    # heap so it can be closed (LIFO) mid-emission to make room for wo.
    x_ctx = ExitStack()
    x_pool = x_ctx.enter_context(tc.tile_pool(name="xp", bufs=1, side="left"))
    X = x_pool.tile([128, CT, S], BF16)           # [hid%128, ct, tok]

    w_pool = ctx.enter_context(tc.tile_pool(name="wp", bufs=2, side="right"))
    st_pool = ctx.enter_context(tc.tile_pool(name="st", bufs=1, side="right"))
    e_pool = ctx.enter_context(tc.tile_pool(name="ep", bufs=3, side="right"))
    rz_pool = ctx.enter_context(tc.tile_pool(name="rz", bufs=1, side="right"))

    # Persistent psum pool for projection chunks: 1 bank, used by the
    # q-projection filler during the slots as well.
    pq_ctx = ExitStack()
    p_q = pq_ctx.enter_context(
        tc.tile_pool(name="p_q", bufs=1, space="PSUM", side="left"))

    def rope(ps, c0, n, dst):
        """RoPE a [128, n] psum tile (layout [d, pos], positions c0:c0+n)
        -> bf16 SBUF dst. rotate_half is a cross-partition half-swap; the
        sign lives in SINM (rows 0:64 pre-negated on the host)."""
        tmp = st_pool.tile([128, 512], F32, tag="tmp", bufs=2, name="tmp")
        stage = st_pool.tile([128, 512], F32, tag="stage", bufs=2, name="stage")
        nc.vector.tensor_mul(stage[0:64, 0:n], ps[64:128, :],
                             SINM[0:64, c0:c0 + n])
        nc.vector.tensor_mul(stage[64:128, 0:n], ps[0:64, :],
                             SINM[64:128, c0:c0 + n])
        nc.vector.tensor_mul(tmp[:, 0:n], ps[:], COS[:, c0:c0 + n])
        nc.vector.tensor_add(dst, stage[:, 0:n], tmp[:, 0:n])

    # ---------------- preamble: K, V, Q(0) projections ----------------
    # DMA priority: feed the PE within ~4.5us and keep the X stream just
    # ahead of the 256-token preamble chunk walk (6.8us compute / 2.9us
    # DMA per chunk).
    wk_ts = []
    wk_t0 = w_pool.tile([128, CT, 128], BF16, tag="w", bufs=3, name="wk_t")
    nc.sync.dma_start(wk_t0[:], wkt_d[0])
    wk_ts.append(wk_t0)
    nc.sync.dma_start(X[:, :, 0:256], xt_d[:, :, 0:256])
    nc.scalar.dma_start(COS[:, 0:512], cosk_d[:, 0:512])
    nc.scalar.dma_start(SINM[:, 0:512], sinkm_d[:, 0:512])
    wk_t1 = w_pool.tile([128, CT, 128], BF16, tag="w", bufs=3, name="wk_t")
    nc.sync.dma_start(wk_t1[:], wkt_d[1])
    wk_ts.append(wk_t1)
    wv_t = w_pool.tile([128, CT, 256], BF16, tag="wv", bufs=1, name="wv_t")
    nc.sync.dma_start(wv_t[:], wvt_d[:])
    nc.sync.dma_start(X[:, :, 256:512], xt_d[:, :, 256:512])
    nc.scalar.dma_start(COS[:, 512:1024], cosk_d[:, 512:1024])
    nc.scalar.dma_start(SINM[:, 512:1024], sinkm_d[:, 512:1024])
    nc.sync.dma_start(X[:, :, 512:768], xt_d[:, :, 512:768])
    nc.scalar.dma_start(COS[:, 1024:2048], cosk_d[:, 1024:2048])
    nc.scalar.dma_start(SINM[:, 1024:2048], sinkm_d[:, 1024:2048])
    for tch in range(3, 8):
        t0 = tch * 256
        nc.sync.dma_start(X[:, :, t0:t0 + 256], xt_d[:, :, t0:t0 + 256])
    wq_t0 = w_pool.tile([128, CT, 128], BF16, tag="w", bufs=3, name="wq_t")
    nc.sync.dma_start(wq_t0[:], wqt_d[0])
    nc.sync.dma_start(ones_t[:], ones_d[:])
    nc.sync.dma_start(ident_t[:], ident_d[:])

    with tc.tile_pool(name="p_pre", bufs=1, space="PSUM", side="right") as p_pre:
        for tch in range(8):
            j0 = tch * 256
            for g in range(KVC):
                ps = p_pre.tile([128, 512], F32, tag="c", bufs=6, name="ps_pre")
                for ct in range(CT):
                    nc.tensor.matmul(ps[:, 0:256], wk_ts[g][:, ct, :],
                                     X[:, ct, j0:j0 + 256],
                                     start=(ct == 0), stop=(ct == CT - 1))
                rope(ps[:, 0:256], j0, 256, KT[:, g, j0:j0 + 256])
            for jt in range(tch * 2, tch * 2 + 2):
                ps = p_pre.tile([128, 512], F32, tag="c", bufs=6, name="ps_pre")
                for ct in range(CT):
                    nc.tensor.matmul(ps[:, 0:256],
                                     X[:, ct, jt * 128:(jt + 1) * 128],
                                     wv_t[:, ct, :],
                                     start=(ct == 0), stop=(ct == CT - 1))
                nc.scalar.copy(VV[:, jt, :], ps[:, 0:256])
        # Q projection head 0.
        for qc in range(4):
            i0 = qc * 512
            ps = p_pre.tile([128, 512], F32, tag="c", bufs=6, name="ps_pre")
            for ct in range(CT):
                nc.tensor.matmul(ps[:], wq_t0[:, ct, :],
                                 X[:, ct, i0:i0 + 512],
                                 start=(ct == 0), stop=(ct == CT - 1))
            rope(ps, i0, 512, QT[:, 0, i0:i0 + 512])

    # ---------------- q-projection filler machinery ----------------
    filler = deque()

    wq_pref = {}

    def prefetch_wq(h):
        if h < HC and h not in wq_pref:
            wq_t = w_pool.tile([128, CT, 128], BF16, tag="w", bufs=3,
                               name="wq_t")
            nc.sync.dma_start(wq_t[:], wqt_d[h])
            wq_pref[h] = wq_t

    def push_qproj(h):
        """Queue head h's q-projection as small PE filler pieces."""
        prefetch_wq(h)
        state = {"w": wq_pref.pop(h)}

        for qc in range(4):
            def open_chunk(qc=qc):
                ps = p_q.tile([128, 512], F32, tag="q", bufs=1, name="ps_q")
                state["ps"] = ps
            filler.append((False, open_chunk))
            for c4 in range(4):
                def mm_piece(qc=qc, c4=c4):
                    ps = state["ps"]
                    for ct in range(c4 * 4, c4 * 4 + 4):
                        nc.tensor.matmul(
                            ps[:], state["w"][:, ct, :],
                            X[:, ct, qc * 512:qc * 512 + 512],
                            start=(ct == 0), stop=(ct == CT - 1))
                filler.append((True, mm_piece))

            def rope_piece(qc=qc):
                rope(state["ps"], qc * 512, 512, QT[:, h, qc * 512:qc * 512 + 512])
            filler.append((False, rope_piece))

    # Fixed-position pacing: a qproj chunk is 6 pieces (open, 4 mm, rope)
    # emitted at jts {0..4, 8} and {8..12, ...} of each slot, i.e. two
    # chunks per slot, finishing well clear of the slot boundary so the
    # single qproj psum bank and the DVE rope never collide with the
    # slot-end usb/normalize chain.
    PIECE_POINTS = (0, 1, 2, 3, 4, 5, 8, 9, 10, 11, 12, 13)

    def drain_at(point):
        k = 0
        while filler and k < PIECE_POINTS.count(point):
            filler.popleft()[1]()
            k += 1

    def flush_filler():
        while filler:
            filler.popleft()[1]()

    # ---------------- attention slots ----------------
    wo_ts = []
    prefilled = set()
    ys_ev = [0]

    def oproj_group(key):
        tt, ob = key
        o0 = ob * 512
        ps = p_q.tile([128, 512], F32, tag="q", bufs=1, name="ps_q")
        for hh in range(HC):
            nc.tensor.matmul(ps[:], OT[:, hh, tt * 128:(tt + 1) * 128],
                             wo_ts[hh][:, o0:o0 + 512],
                             start=(hh == 0), stop=(hh == HC - 1))
        yt = st_pool.tile([128, 512], F32, tag="yt8", bufs=2, name="yt")
        if ys_ev[0] % 2 == 0:
            nc.scalar.copy(yt[:], ps[:])
        else:
            nc.vector.tensor_copy(yt[:], ps[:])
        ys_ev[0] += 1
        nc.sync.dma_start(y_d[tt * 128:(tt + 1) * 128, o0:o0 + 512], yt[:])
        prefilled.add(key)

    og_iter = iter([(0, 0), (0, 1), (0, 2), (0, 3), (1, 0)])
    with (
        tc.tile_pool(name="p_s", bufs=1, space="PSUM", side="right") as p_s,
        tc.tile_pool(name="p_u", bufs=1, space="PSUM", side="right") as p_u,
        tc.tile_pool(name="p_z", bufs=1, space="PSUM", side="right") as p_z,
    ):
        def make_slot(h, u):
            return {
                "h": h, "u": u, "g": h // 4, "i0": u * 1024,
                "U": p_u.tile([128, 1024], F32, tag="U", bufs=1, name="ps_U"),
                "Z": p_z.tile([128, 512], F32, tag="Z", bufs=1, name="ps_Z"),
                "Es": {}, "zred": [None, None],
            }

        def s_score(st, jt):
            ps = p_s.tile([128, 1024], F32, tag="S", bufs=2, name="ps_S")
            kt_sl = KT[:, st["g"], jt * 128:(jt + 1) * 128]
            i0 = st["i0"]
            nc.tensor.matmul(ps[:, 0:512], kt_sl,
                             QT[:, st["h"], i0:i0 + 512],
                             start=True, stop=True)
            nc.tensor.matmul(ps[:, 512:1024], kt_sl,
                             QT[:, st["h"], i0 + 512:i0 + 1024],
                             start=True, stop=True)
            E = e_pool.tile([128, 1024], BF16, tag="e", bufs=4, name="E")
            nc.scalar.activation(E[:], ps[:], Exp, scale=SCALE)
            st["Es"][jt] = E

        def s_av_z(st, jt):
            # Z column layout: [jt-half, ib, jt%8] so the first half's
            # reduce can run mid-slot and the next slot's writes never
            # collide with this slot's late reduce.
            E = st["Es"].pop(jt)
            v_sl = VV[:, jt, st["g"] * 128:(st["g"] + 1) * 128]
            s0, sp = (jt == 0), (jt == JT - 1)
            U_ps, Z_ps = st["U"], st["Z"]
            nc.tensor.matmul(U_ps[:, 0:512], v_sl, E[:, 0:512],
                             start=s0, stop=sp)
            nc.tensor.matmul(U_ps[:, 512:1024], v_sl, E[:, 512:1024],
                             start=s0, stop=sp)
            col0 = (jt // 8) * 64 + (jt % 8)
            for ib in range(8):
                nc.tensor.matmul(
                    Z_ps[:, col0 + ib * 8:col0 + ib * 8 + 1],
                    E[:, ib * 128:(ib + 1) * 128], ones_t[:],
                    start=True, stop=True)

        def s_reduce_half(st, half):
            zr = rz_pool.tile([128, 8], F32, tag=f"zred{half}", bufs=1,
                              name="zred")
            nc.vector.tensor_reduce(
                zr[:],
                st["Z"][:, half * 64:half * 64 + 64].rearrange(
                    "p (a b) -> p a b", a=8),
                axis=AxX, op=Add)
            st["zred"][half] = zr

        def s_ztail(st):
            h, i0, U_ps, Z_ps = st["h"], st["i0"], st["U"], st["Z"]
            # Evict U to SBUF right away so the U psum bank is free
            # before the next slot's first AV matmul.
            # softmax denominator first on DVE (short ops) so the PE
            # transpose is unblocked before the 1.2us usb eviction runs.
            s_reduce_half(st, 1)
            zs = rz_pool.tile([128, 8], F32, tag="zs", bufs=1, name="zs")
            nc.vector.tensor_add(zs[:], st["zred"][0][:], st["zred"][1][:])
            rz = rz_pool.tile([128, 8], F32, tag="rz", bufs=1, name="rz")
            nc.vector.reciprocal(rz[:], zs[:])
            usb = rz_pool.tile([128, 1024], BF16, tag="usb", bufs=1,
                               name="usb")
            nc.vector.tensor_copy(usb[:], U_ps[:])
            nc.tensor.matmul(Z_ps[0:8, 256:384], rz[:], ident_t[:],
                             start=True, stop=True, is_transpose=True)
            rzt = rz_pool.tile([8, 128], BF16, tag="rzt", bufs=1, name="rzt")
            nc.vector.tensor_copy(rzt[:], Z_ps[0:8, 256:384])
            # flatten [8,128] -> [1,1024] via a DRAM round trip (plain
            # DMAs, off the critical path), then one gpsimd broadcast.
            rsc = dram_pool.tile([8, 128], BF16, tag="rsc", bufs=2,
                                 name="rsc")
            nc.sync.dma_start(rsc[:], rzt[:])
            rzrow = rz_pool.tile([1, 1024], BF16, tag="rzrow", bufs=1,
                                 name="rzrow")
            nc.sync.dma_start(
                rzrow[:], rsc.rearrange("a b -> (a b)").unsqueeze(0))
            rzf = rz_pool.tile([128, 1024], BF16, tag="rzf", bufs=1,
                               name="rzf")
            nc.gpsimd.partition_broadcast(rzf[:], rzrow[0:1, :])
            nc.gpsimd.tensor_mul(OT[:, h, i0:i0 + 1024], usb[:], rzf[:])

        slots = [(h, u) for h in range(HC) for u in range(2)]
        cur = make_slot(0, 0)
        s_score(cur, 0)
        s_score(cur, 1)
        for idx, (h, u) in enumerate(slots):
            if u == 0:
                prefetch_wq(h + 2)
                if h + 1 < HC:
                    push_qproj(h + 1)
            st = cur
            for jt in range(JT):
                s_av_z(st, jt)
                if jt + 2 < JT:
                    s_score(st, jt + 2)
                if jt == 8:
                    s_reduce_half(st, 0)
                if jt in PIECE_POINTS:
                    drain_at(jt)
                if h == HC - 1 and u == 1 and jt in (3, 6, 9, 12, 14):
                    oproj_group(next(og_iter))
            if idx + 1 < len(slots):
                # Pipeline the slot front: the next slot's first two
                # scores go ahead of this slot's normalize tail so ACT
                # starts its exps ~1.5us earlier and the next AV(0)
                # never waits.
                cur = make_slot(*slots[idx + 1])
                s_score(cur, 0)
                s_score(cur, 1)
            s_ztail(st)
            if h == HC - 2 and u == 1:
                # X no longer needed (last q-projection emitted); swap the
                # X heap space for the wo tiles.
                flush_filler()
                x_ctx.close()
                wo_pool = ctx.enter_context(
                    tc.tile_pool(name="wo", bufs=1, side="left"))
                for hh in range(HC):
                    wo_t = wo_pool.tile([128, S], BF16, tag=f"wo{hh}",
                                        bufs=1, name="wo_t")
                    nc.sync.dma_start(wo_t[:], wot_d[hh])
                    wo_ts.append(wo_t)

    pq_ctx.close()

    # ---------------- output projection ----------------
    with (
        tc.tile_pool(name="p_y", bufs=1, space="PSUM", side="right") as p_y,
        tc.tile_pool(name="ys", bufs=1, side="right") as ys_pool,
    ):
        ev = 0
        for tt in range(16):
            for ob in range(4):
                if (tt, ob) in prefilled:
                    continue
                o0 = ob * 512
                ps = p_y.tile([128, 512], F32, tag="y", bufs=8, name="ps_y")
                for h in range(HC):
                    nc.tensor.matmul(ps[:], OT[:, h, tt * 128:(tt + 1) * 128],
                                     wo_ts[h][:, o0:o0 + 512],
                                     start=(h == 0), stop=(h == HC - 1))
                yt = ys_pool.tile([128, 512], F32, tag="yt", bufs=8, name="yt")
                if ev % 2 == 0:
                    nc.scalar.copy(yt[:], ps[:])
                else:
                    nc.vector.tensor_copy(yt[:], ps[:])
                ev += 1
                nc.sync.dma_start(y_d[tt * 128:(tt + 1) * 128, o0:o0 + 512],
                                  yt[:])

    ctx.close()


def _build(repeat=1):
    import concourse.mybir as mybir
    import concourse.tile as tile
    from concourse import bacc

    F32 = mybir.dt.float32
    BF16 = mybir.dt.bfloat16

    nc = bacc.Bacc("TRN2", target_bir_lowering=False, debug=False)
    xt_d = nc.dram_tensor("xt", [128, CT, S], BF16, kind="ExternalInput").ap()
    cosk_d = nc.dram_tensor("cosk", [128, S], BF16, kind="ExternalInput").ap()
    sinkm_d = nc.dram_tensor("sinkm", [128, S], BF16,
                             kind="ExternalInput").ap()
    wqt_d = nc.dram_tensor("wqt", [HC, 128, CT, 128], BF16,
                           kind="ExternalInput").ap()
    wkt_d = nc.dram_tensor("wkt", [KVC, 128, CT, 128], BF16,
                           kind="ExternalInput").ap()
    wvt_d = nc.dram_tensor("wvt", [128, CT, 256], BF16,
                           kind="ExternalInput").ap()
    wot_d = nc.dram_tensor("wot", [HC, 128, S], BF16, kind="ExternalInput").ap()
    ones_d = nc.dram_tensor("ones", [128, 1], BF16, kind="ExternalInput").ap()
    ident_d = nc.dram_tensor("ident", [128, 128], F32,
                             kind="ExternalInput").ap()
    y_d = nc.dram_tensor("y", [S, HID], F32, kind="ExternalOutput").ap()

    with tile.TileContext(nc) as tc:
        for _ in range(repeat):
            _emit(nc, tc, (xt_d, cosk_d, sinkm_d, wqt_d, wkt_d, wvt_d, wot_d,
                           ones_d, ident_d, y_d))
    nc.compile()
    return nc


class _Runner:
    """Persistent-jit PJRT executor (axon) / NRT executor (native)."""

    def __init__(self, nc):
        self.nc = nc
        from concourse._compat import axon_active
        self.axon = axon_active()
        if not self.axon:
            return
        import jax
        from jax.sharding import Mesh, PartitionSpec
        from jax.experimental.shard_map import shard_map
        import concourse.mybir as mybir
        from concourse.bass2jax import (
            _bass_exec_p, install_neuronx_cc_hook, partition_id_tensor)

        install_neuronx_cc_hook()
        partition_name = (nc.partition_id_tensor.name
                          if nc.partition_id_tensor else None)
        in_names, out_names, out_avals, zero_outs = [], [], [], []
        for alloc in nc.m.functions[0].allocations:
            if not isinstance(alloc, mybir.MemoryLocationSet):
                continue
            name = alloc.memorylocations[0].name
            if alloc.kind == "ExternalInput":
                if name != partition_name:
                    in_names.append(name)
            elif alloc.kind == "ExternalOutput":
                shape = tuple(alloc.tensor_shape)
                dtype = mybir.dt.np(alloc.dtype)
                out_names.append(name)
                out_avals.append(jax.core.ShapedArray(shape, dtype))
                zero_outs.append(np.zeros(shape, dtype))
        self.in_names, self.out_names = in_names, out_names
        self.zero_outs = zero_outs
        n_params, n_outs = len(in_names), len(out_names)
        all_in = in_names + out_names
        if partition_name is not None:
            all_in.append(partition_name)
        donate = tuple(range(n_params, n_params + n_outs))

        def _body(*args):
            operands = list(args)
            if partition_name is not None:
                operands.append(partition_id_tensor())
            return tuple(_bass_exec_p.bind(
                *operands,
                out_avals=tuple(out_avals),
                in_names=tuple(all_in),
                out_names=tuple(out_names),
                lowering_input_output_aliases=(),
                sim_require_finite=True,
                sim_require_nnan=True,
                nc=nc,
            ))

        devices = jax.devices()[:N_CORES]
        mesh = Mesh(np.asarray(devices), ("core",))
        self._fn = jax.jit(
            shard_map(_body, mesh=mesh,
                      in_specs=(PartitionSpec("core"),) * (n_params + n_outs),
                      out_specs=(PartitionSpec("core"),) * n_outs,
                      check_rep=False),
            donate_argnums=donate, keep_unused=True,
        )

    def run(self, in_maps):
        if not self.axon:
            from concourse import bass_utils
            res = bass_utils.run_bass_kernel_spmd(
                self.nc, in_maps, core_ids=list(range(N_CORES)))
            return res.results
        concat_in = [
            np.concatenate([np.asarray(in_maps[c][n]) for c in range(N_CORES)],
                           axis=0)
            for n in self.in_names
        ] + [np.concatenate([z] * N_CORES, axis=0) for z in self.zero_outs]
        outs = [np.asarray(o) for o in self._fn(*concat_in)]
        per_core = []
        for c in range(N_CORES):
            d = {}
            for name, o in zip(self.out_names, outs):
                rows = o.shape[0] // N_CORES
                d[name] = o[c * rows:(c + 1) * rows]
            per_core.append(d)
        return per_core


def _prep_inputs(x, cos, sin, wq, wk, wv, wo):
    import concourse.mybir as mybir
    f32 = np.float32
    bf16 = mybir.dt.np(mybir.dt.bfloat16)

    cosT = np.asarray(cos).T.astype(f32)    # [128, S]
    sinm = np.asarray(sin).T.astype(f32).copy()
    sinm[0:64] *= -1.0
    cosT = np.ascontiguousarray(cosT).astype(bf16)
    sinm = np.ascontiguousarray(sinm).astype(bf16)
    ones = np.ones((128, 1), bf16)
    ident = np.eye(128, dtype=f32)
    x = np.asarray(x, f32)
    wq = np.asarray(wq, f32)
    wk = np.asarray(wk, f32)
    wv = np.asarray(wv, f32)
    wo = np.asarray(wo, f32)

    in_maps = []
    for c in range(N_CORES):
        b, kh = c // 2, c % 2
        # X^T packed [p, ct, tok]: [p, ct, j] = x[b, j, ct*128+p]
        xt = np.ascontiguousarray(
            x[b].T.reshape(CT, 128, S).transpose(1, 0, 2)).astype(bf16)
        # wq rows for this core's heads -> [h, p(ct-part), ct, c(col)]
        wq_c = wq[kh * 1024:(kh + 1) * 1024, :]
        wqt = np.ascontiguousarray(
            wq_c.reshape(HC, 128, CT, 128).transpose(0, 3, 2, 1)).astype(bf16)
        wk_c = wk[kh * 256:(kh + 1) * 256, :]
        wkt = np.ascontiguousarray(
            wk_c.reshape(KVC, 128, CT, 128).transpose(0, 3, 2, 1)).astype(bf16)
        wv_c = wv[kh * 256:(kh + 1) * 256, :]
        wvt = np.ascontiguousarray(
            wv_c.reshape(256, CT, 128).transpose(2, 1, 0)).astype(bf16)
        # wo columns for this core's heads -> [h, p(=d), out]
        wot = np.ascontiguousarray(
            wo[:, kh * 1024:(kh + 1) * 1024].T.reshape(HC, 128, S)).astype(bf16)
        in_maps.append({
            "xt": xt, "cosk": cosT, "sinkm": sinm,
            "wqt": wqt, "wkt": wkt, "wvt": wvt, "wot": wot,
            "ones": ones, "ident": ident,
        })
    return in_maps


def kernel(x, cos, sin, wq, wk, wv, wo):
    if "nc" not in _cache:
        _cache["nc"] = _build()
        _cache["runner"] = _Runner(_cache["nc"])
    runner = _cache["runner"]
    in_maps = _prep_inputs(x, cos, sin, wq, wk, wv, wo)
    results = runner.run(in_maps)
    y = np.empty((B, S, HID), np.float32)
    for b in range(B):
        y[b] = (results[2 * b]["y"].astype(np.float32)
                + results[2 * b + 1]["y"].astype(np.float32))
    return y
